# revision 53
# baseline (speedup 1.0000x reference)
"""RNN-T joint network (dense MLP) Trainium2 Bass kernel.

Math (per batch row n):
    h = relu(f @ W1t.T + g @ W1p.T + b1t + b1p)    # [N, 512]
    y = h @ W2.T + b2                              # [N, 29]

Strategy: data-parallel over batch N=32768 across 8 NeuronCores (4096
rows/core); weights replicated.

Layer 1 runs on the PE in fp8e4m3 DoubleRow mode (2 k-tiles per matmul,
0.5 cyc/row per the TRN2 cost model) with a hi+lo residual decomposition
of BOTH operands to recover accuracy:
    x ~= xh + xl,  W*32 ~= wh + wl   (each term e4m3)
    h*32 ~= xh@wh + xl@wh + xh@wl    (xl@wl is negligible; the xh@wl
    correction also skips K rows 1280..1343 -- see LHS0/RHS0 below)
Host stacks xc = [xh; xl] as 21 exact 128-row k-tiles ([2688, 4096]).
The 3 products become 16 DoubleRow matmuls per (j-tile, chunk) driven
by the LHS0/RHS0 slot tables against a weight-slot tensor
w1q[128, 22, JOINT_H].  Measured end-to-end relative error: 8.6e-3
(gate: 2e-2).  Dequant scale 1/32 + bias + ReLU fold into the ScalarE
activation, which writes h in fp32; layer 2 stays fp32r (4 matmuls of
512 rows per chunk) as in the fp32r baseline.

Schedule: weights ride the SP/HWDGE DMA path while x rides the
Pool/SWDGE path (parallel issue); chunks 0-1 emit DR-pair-outer so the
PE starts on partially-arrived data; each item's last three L2 matmuls
+ y-act + y-DMA are software-pipelined into the next item; the final
chunk is computed as two 256-col halves to shrink the drain tail.

PE cost per core: 16*256*32 + 4*512*8 = 147.5k cycles ~= 61.4us, vs
196.6k (86.4us measured) for the fp32r baseline.  DMA drops from 23MB
to ~12.5MB.  TimelineSim: 72.5us vs 98.4us baseline.
"""

import numpy as np
import ml_dtypes

import concourse.bacc as bacc
import concourse.bass as bass  # noqa: F401
import concourse.mybir as mybir
from concourse import tile
from concourse.bass_utils import run_bass_kernel_spmd

TRANS_H, PRED_H, JOINT_H, NUM_LABELS = 1024, 320, 512, 29
BATCH = 32768
N_CORES = 8
N_PER_CORE = BATCH // N_CORES          # 4096
K_TOTAL = TRANS_H + PRED_H             # 1344
KC = 2688                              # 21 x-tiles: see layout below
KT = KC // 128                         # 21
N_DR = 16                              # DoubleRow matmuls per (j, chunk)
W_SLOTS = 22                           # weight slots (wh deduplicated)
J_TILES = JOINT_H // 128               # 4
N_CHUNK = 512                          # PSUM-bank limit for fp32 [128, 512]
N_CHUNKS = N_PER_CORE // N_CHUNK       # 8
S1 = 32.0                              # W1 quant scale (dequant in ACT)

F32 = mybir.dt.float32
F32R = mybir.dt.float32r
F8 = mybir.dt.float8e4
E4 = ml_dtypes.float8_e4m3fn

# xc tile layout (21 tiles of 128 rows, no padding):
#   tiles 0..9  = xh rows 0..1279
#   tiles 10..19 = xl rows 0..1279
#   tile 20     = xh rows 1280..1343 (p 0..63) + xl rows 1280..1343 (p 64..127)
# weight slot layout (22 slots), interleaved in chunk-0 consumption order:
#   slots 4t, 4t+1   = wl tiles 2t, 2t+1   (t = 0..4, B sweep)
#   slots 4t+2, 4t+3 = wh tiles 2t, 2t+1   (shared by A-hi and A-lo sweeps)
#   slot 20          = ZERO (memset on-chip, never DMA'd)
#   slot 21          = wh rows 1280..1343 stacked twice (p 0..63 and 64..127)
# DR matmul d contracts lhsT slots (LHS0[d], LHS0[d]+1) against rhs x-tiles
# (RHS0[d], RHS0[d]+1):
#   d 0..4:  B sweep  xh @ wl        d 5..9:  A-hi sweep xh @ wh
#   d 10..14: A-lo sweep xl @ wh     d 15: A leftovers
#   (d 15 pairs a zero slot against a duplicated rhs tile 19 and handles
#    BOTH xh_t10 @ wh10 and xl_t10 @ wh10 via the stacked tile 20;
#    the B correction intentionally skips rows 1280..1343 -- correcting
#    64/1344 rows costs a full 256-cycle DR matmul for a measured rel-err
#    change of only 7.7e-3 -> 1.1e-3, both far under the 2e-2 gate)
LHS0 = ([4 * d for d in range(5)] + [4 * d + 2 for d in range(5)]
        + [4 * d + 2 for d in range(5)] + [20])
RHS0 = ([2 * d for d in range(5)] + [2 * d for d in range(5)]
        + [10 + 2 * d for d in range(5)] + [19])
assert len(RHS0) == len(LHS0) == N_DR

_NC_CACHE = {}


def _build_bass():
    """Build the single-core Bass program (same NEFF runs SPMD on 8 cores)."""
    nc = bacc.Bacc(None)

    xc = nc.dram_tensor("xc", [KC, N_PER_CORE], F8, kind="ExternalInput")
    w1q = nc.dram_tensor("w1q", [128, W_SLOTS, JOINT_H], F8, kind="ExternalInput")
    b1 = nc.dram_tensor("b1", [JOINT_H, 1], F32, kind="ExternalInput")
    w2T = nc.dram_tensor("w2T", [JOINT_H, NUM_LABELS], F32R, kind="ExternalInput")
    b2 = nc.dram_tensor("b2", [NUM_LABELS, 1], F32, kind="ExternalInput")
    yT = nc.dram_tensor("yT", [NUM_LABELS, N_PER_CORE], F32, kind="ExternalOutput")

    # view with the k-tile index explicit: row (k*128 + p) -> [p, k, n]
    xc3 = xc.rearrange("(k p) n -> p k n", p=128)      # [128, KT, N]

    # steady-state x DMA pieces (chunks 2+)
    X_SPLITS = [(0, 11), (11, KT)]

    # chunk-0 emission order: interleave the B and A-hi sweeps (they read
    # the same x tiles), so each newly-landed DMA piece feeds two matmul
    # pairs instead of one while the fill streams in
    ORDER_FILL = [0, 5, 1, 6, 2, 7, 3, 8, 4, 9, 10, 11, 12, 13, 14, 15]
    assert sorted(ORDER_FILL) == list(range(N_DR))

    with tile.TileContext(nc) as tc:
        with (
            tc.tile_pool(name="consts", bufs=1) as consts,
            tc.tile_pool(name="xpool", bufs=3) as xpool,
            tc.tile_pool(name="hpool", bufs=2) as hpool,
            tc.tile_pool(name="opool", bufs=2) as opool,
            tc.tile_pool(name="psum_h", bufs=6, space="PSUM") as psum_h,
            tc.tile_pool(name="psum_y", bufs=2, space="PSUM") as psum_y,
        ):
            # ---- fill: fp8 weights + chunk-0 x, ordered by first use; the
            # sync (HWDGE) and gpsimd (Pool/SWDGE) issue paths run in
            # parallel, so pieces alternate between them roughly in the
            # order the chunk-0 pair-outer matmuls consume them
            w1_sb = consts.tile([128, W_SLOTS, JOINT_H], F8, name="w1_sb",
                                tag="w1")
            x0_sb = xpool.tile([128, KT, N_CHUNK], F8, name="x_sb", tag="x")

            def dma_w(a, b):
                nc.sync.dma_start(out=w1_sb[:, a:b, :], in_=w1q[:, a:b, :])

            # zero slot 20 on-chip instead of shipping 64KB of zeros
            nc.vector.memset(w1_sb[:, 20:21, :], 0)

            def dma_x0(eng, a, b):
                eng.dma_start(out=x0_sb[:, a:b, :], in_=xc3[:, a:b, 0:N_CHUNK])

            x1_sb = xpool.tile([128, KT, N_CHUNK], F8, name="x_sb", tag="x")

            def dma_x1(a, b):
                nc.gpsimd.dma_start(
                    out=x1_sb[:, a:b, :], in_=xc3[:, a:b, N_CHUNK:2 * N_CHUNK]
                )

            dma_w(0, 2)
            dma_x0(nc.gpsimd, 0, 2)
            dma_w(2, 6)
            dma_x0(nc.gpsimd, 2, 6)
            dma_x0(nc.gpsimd, 6, 10)
            dma_w(6, 14)
            dma_x0(nc.gpsimd, 10, 14)
            dma_w(14, 20)
            dma_x0(nc.gpsimd, 14, 18)
            dma_w(21, W_SLOTS)
            # chunk 1's early x pieces beat chunk 0's last piece onto the
            # DMA engines: c0 only needs tiles 18..21 for its final DRs,
            # and chunk 1 runs pair-outer so it can start on partial data
            dma_x1(0, 4)
            dma_x0(nc.gpsimd, 18, KT)
            dma_x1(4, 10)
            dma_x1(10, 14)
            dma_x1(14, 18)
            dma_x1(18, KT)

            # ---- small replicated constants (needed from the first ACT /
            # L2, ~6us in, so they queue behind the critical fill pieces) ----
            b1_sb = consts.tile([128, J_TILES], F32, name="b1_sb", tag="b1")
            nc.scalar.dma_start(
                out=b1_sb, in_=b1.rearrange("(j p) o -> p (j o)", p=128)
            )
            w2_sb = consts.tile([128, J_TILES, NUM_LABELS], F32R, name="w2_sb",
                                tag="w2")
            nc.scalar.dma_start(
                out=w2_sb, in_=w2T.rearrange("(j p) l -> p j l", p=128)
            )
            b2_sb = consts.tile([NUM_LABELS, 1], F32, name="b2_sb", tag="b2")
            nc.scalar.dma_start(out=b2_sb, in_=b2[:, :])

            # ---- main loop ----
            # work item = (chunk, col offset in chunk, width): the last
            # chunk is computed as two 256-col halves so the final serial
            # L1 -> ACT -> L2 -> ACT -> DMA tail is half-width; its x DMA
            # is still one 512-col transfer (keeps 512B DMA elements)
            work = [(c, 0, N_CHUNK) for c in range(N_CHUNKS - 1)]
            work += [(N_CHUNKS - 1, 0, 256), (N_CHUNKS - 1, 256, 256)]

            x_tiles = {0: x0_sb, 1: x1_sb}
            deferred = None  # emits previous item's l2(3) + y-act + y-DMA

            def make_item(c, o, w, x_sb, pair_outer, after_first_block):
                n0 = c * N_CHUNK + o
                phs = [
                    psum_h.tile([128, w], F32, name=f"ph_{j}", tag="ph")
                    for j in range(J_TILES)
                ]
                py = psum_y.tile([NUM_LABELS, w], F32, name="py", tag="py")
                h_tiles = {}

                def l1_block(j):
                    for d in range(N_DR):
                        nc.tensor.matmul(
                            phs[j],
                            lhsT=w1_sb[:, LHS0[d]:LHS0[d] + 2,
                                       j * 128:(j + 1) * 128],
                            rhs=x_sb[:, RHS0[d]:RHS0[d] + 2, o:o + w],
                            start=(d == 0),
                            stop=(d == N_DR - 1),
                            perf_mode=mybir.MatmulPerfMode.DoubleRow,
                        )

                def l1_block_pair_outer():
                    for idx, d in enumerate(ORDER_FILL):
                        for j in range(J_TILES):
                            nc.tensor.matmul(
                                phs[j],
                                lhsT=w1_sb[:, LHS0[d]:LHS0[d] + 2,
                                           j * 128:(j + 1) * 128],
                                rhs=x_sb[:, RHS0[d]:RHS0[d] + 2, o:o + w],
                                start=(idx == 0),
                                stop=(idx == N_DR - 1),
                                perf_mode=mybir.MatmulPerfMode.DoubleRow,
                            )

                def act_block(j):
                    h_sb = hpool.tile([128, w], F32R, name=f"h_{j}",
                                      tag=f"h_{j}", padded_shape=[128, N_CHUNK])
                    nc.scalar.activation(
                        h_sb, phs[j], mybir.ActivationFunctionType.Relu,
                        bias=b1_sb[:, j:j + 1], scale=1.0 / S1,
                    )
                    h_tiles[j] = h_sb

                def l2_block(j):
                    nc.tensor.matmul(
                        py,
                        lhsT=w2_sb[:, j, :],
                        rhs=h_tiles[j],
                        start=(j == 0),
                        stop=(j == J_TILES - 1),
                    )

                def finish():
                    l2_block(1)
                    l2_block(2)
                    l2_block(3)
                    y_sb = opool.tile([NUM_LABELS, w], F32, name="y_sb",
                                      tag="y")
                    nc.scalar.activation(
                        y_sb, py, mybir.ActivationFunctionType.Identity,
                        bias=b2_sb,
                    )
                    nc.scalar.dma_start(out=yT[:, n0:n0 + w], in_=y_sb)

                if pair_outer:
                    # DR-pair outer, j inner: each newly-landed DMA piece
                    # feeds 4 matmuls while the rest of the fill streams in
                    if after_first_block is not None:
                        after_first_block()
                    l1_block_pair_outer()
                    for j in range(J_TILES):
                        act_block(j)
                    l2_block(0)
                else:
                    l1_block(0)
                    if after_first_block is not None:
                        after_first_block()
                    act_block(0)
                    l1_block(1)
                    act_block(1)
                    l2_block(0)
                    l1_block(2)
                    act_block(2)
                    l1_block(3)
                    act_block(3)
                return finish

            for wi, (c, o, w) in enumerate(work):
                if c not in x_tiles:
                    x_sb = xpool.tile([128, KT, N_CHUNK], F8, name="x_sb",
                                      tag="x")
                    n0 = c * N_CHUNK
                    for (ka, kb) in X_SPLITS:
                        nc.gpsimd.dma_start(
                            out=x_sb[:, ka:kb, :],
                            in_=xc3[:, ka:kb, n0:n0 + N_CHUNK],
                        )
                    x_tiles = {c: x_sb}
                deferred = make_item(c, o, w, x_tiles[c], pair_outer=(wi <= 1),
                                     after_first_block=deferred)
            deferred()

    nc.finalize()
    return nc


def _get_nc():
    if "nc" not in _NC_CACHE:
        _NC_CACHE["nc"] = _build_bass()
    return _NC_CACHE["nc"]


def _quant_hi_lo(a):
    """a (f32) -> (hi, lo) e4m3 with a ~= hi + lo."""
    hi = a.astype(E4)
    lo = (a - hi.astype(np.float32)).astype(E4)
    return hi, lo


def _pack_w1q(W1t, W1p):
    """Per-DR-matmul weight slots [128, W_SLOTS, JOINT_H] (e4m3)."""
    W1 = np.concatenate(
        [np.asarray(W1t, np.float32).T, np.asarray(W1p, np.float32).T], axis=0
    )                                            # [1344, 512]
    wh, wl = _quant_hi_lo(W1 * S1)
    whf = wh.astype(np.float32)
    wlf = wl.astype(np.float32)

    slots = np.zeros((W_SLOTS, 128, JOINT_H), np.float32)
    for t in range(5):
        slots[4 * t] = wlf[256 * t:256 * t + 128]         # B sweep
        slots[4 * t + 1] = wlf[256 * t + 128:256 * t + 256]
        slots[4 * t + 2] = whf[256 * t:256 * t + 128]     # A-hi / A-lo sweeps
        slots[4 * t + 3] = whf[256 * t + 128:256 * t + 256]
    slots[21][0:64] = whf[1280:1344]                 # A leftovers: xh_t10 and
    slots[21][64:128] = whf[1280:1344]               # xl_t10 share xc tile 20
    return np.ascontiguousarray(
        slots.astype(E4).transpose(1, 0, 2)
    )                                            # [128, W_SLOTS, 512]


def _prep_in_maps(f, g, W1t, b1t, W1p, b1p, W2, b2):
    f2 = np.asarray(f, np.float32).reshape(BATCH, TRANS_H)
    g2 = np.asarray(g, np.float32).reshape(BATCH, PRED_H)

    w1q = _pack_w1q(W1t, W1p)
    b1 = (np.asarray(b1t, np.float32) + np.asarray(b1p, np.float32)).reshape(
        JOINT_H, 1
    )
    w2T = np.ascontiguousarray(np.asarray(W2, np.float32).T)
    b2c = np.asarray(b2, np.float32).reshape(NUM_LABELS, 1)

    in_maps = []
    for core in range(N_CORES):
        sl = slice(core * N_PER_CORE, (core + 1) * N_PER_CORE)
        xT = np.empty((K_TOTAL, N_PER_CORE), np.float32)
        xT[:TRANS_H] = f2[sl].T
        xT[TRANS_H:] = g2[sl].T
        xh, xl = _quant_hi_lo(xT)
        xc = np.empty((KC, N_PER_CORE), E4)      # [2688, 4096] e4m3
        xc[0:1280] = xh[0:1280]
        xc[1280:2560] = xl[0:1280]
        xc[2560:2624] = xh[1280:1344]            # tile 20: both components
        xc[2624:2688] = xl[1280:1344]
        in_maps.append(
            {"xc": xc, "w1q": w1q, "b1": b1, "w2T": w2T, "b2": b2c}
        )
    return in_maps


def _gather(results):
    y = np.empty((1, BATCH, NUM_LABELS), np.float32)
    for core, r in enumerate(results):
        y[0, core * N_PER_CORE:(core + 1) * N_PER_CORE] = r["yT"].T
    return y


def _run(inputs, trace=False):
    in_maps = _prep_in_maps(
        inputs["f"], inputs["g"], inputs["W1t"], inputs["b1t"],
        inputs["W1p"], inputs["b1p"], inputs["W2"], inputs["b2"],
    )
    res = run_bass_kernel_spmd(
        _get_nc(), in_maps, core_ids=list(range(N_CORES)), trace=trace
    )
    return _gather(res.results), res


def kernel(**inputs) -> np.ndarray:
    out, _ = _run(inputs, trace=False)
    return out
